# revision 18
# baseline (speedup 1.0000x reference)
import sys
for _p in ('/opt/trn_rl_repo',):
    if _p not in sys.path:
        sys.path.insert(0, _p)

"""NLSGCRN cell Bass/Tile kernel for TRN2, batch-sharded SPMD over 8 cores.

Per-core shapes (b_loc = 4 batches):
  x [4,2000,32], state [4,2000,64], x_full [4,12,2000,48], emb [2000,16],
  pools gw/uw/gb/ub/gT/uT, out h [4,2000,64].

v2 structure (vs v1 baseline):
- y-phases loop nch-outer / b-inner: the per-node d-reduction batches all
  4 batches into one op (the e[n,d] scalar is per-partition and shared
  across b), split DVE (d < D_DVE, seeded with bias) / GPSIMD (rest).
- XkT stationaries via PE transposes (identity matmul) into bf16 PSUM,
  evicted by DVE, replacing v1's DRAM-bounce DMA transposes.
- A is spilled to DRAM at generation and its SBUF tile is freed after the
  gate hops; update diffusion streams A chunks back mi-outer with all 16
  output-chunk accumulators resident in PSUM (8 banks exactly).
- PSUM bank budget in y phases: y tiles [128,1024] f32 (2 banks) x2 bufs
  + transpose batch [128,1536] bf16 (2 banks) x2 bufs.
"""

from contextlib import ExitStack

import concourse.bass as bass
import concourse.tile as tile
from concourse import mybir
from concourse._compat import with_exitstack
from concourse.masks import make_identity

F32 = mybir.dt.float32
F32R = mybir.dt.float32r
BF16 = mybir.dt.bfloat16
AF = mybir.ActivationFunctionType
OP = mybir.AluOpType

B_LOC = 4
N = 2000
NCHUNK = 16           # ceil(2000/128)
NFULL = (NCHUNK - 1) * 128   # 1920
NPAD = NCHUNK * 128   # 2048
DIN, DOUT = 32, 64
CIN = 96
CW = 48
WLEN = 12
EMB = 16
K = 3

D_DVE = 8   # d-reduction: first D_DVE iterations on DVE, rest on GPSIMD


def nlen(nch):
    return 128 if nch < NCHUNK - 1 else N - NFULL  # last = 80


def chunked_load(nc, dst, src, eng=None):
    """dst [128, NCHUNK, ...inner] <- src [2000, ...inner] splitting rows."""
    eng = eng or nc.sync
    inner = src.shape[1:]
    eng.dma_start(
        dst[:, 0 : NCHUNK - 1],
        src[0:NFULL].rearrange(
            "(c p) " + " ".join(f"i{j}" for j in range(len(inner)))
            + " -> p c " + " ".join(f"i{j}" for j in range(len(inner))),
            p=128,
        ),
    )
    eng.dma_start(dst[0 : N - NFULL, NCHUNK - 1], src[NFULL:N])


@with_exitstack
def build(ctx: ExitStack, tc: tile.TileContext, io: dict):
    nc = tc.nc

    io = {k: (v[:] if not isinstance(v, bass.AP) else v) for k, v in io.items()}
    x, state, x_full = io["x"], io["state"], io["x_full"]
    emb = io["node_embeddings"]
    out = io["out"]

    const = ctx.enter_context(tc.tile_pool(name="const", bufs=1))
    big = ctx.enter_context(tc.tile_pool(name="big", bufs=1))
    dram = ctx.enter_context(tc.tile_pool(name="dram", bufs=6, space="DRAM"))

    # ================= constants / weights =================
    eexp = const.tile([128, NCHUNK, EMB], BF16)
    nc.vector.memset(eexp[:], 0.0)
    chunked_load(nc, eexp, emb, eng=nc.gpsimd)

    ident = const.tile([128, 128], BF16)
    make_identity(nc, ident)

    biasg = const.tile([128, NCHUNK, 2 * DOUT], BF16)
    biasu = const.tile([128, NCHUNK, DOUT], BF16)
    nc.vector.memset(biasg[64:, NCHUNK - 1], 0.0)
    nc.vector.memset(biasu[64:, NCHUNK - 1], 0.0)
    rinv = const.tile([128, NCHUNK], F32)
    dsum_all = const.tile([128, NCHUNK], F32)

    # WPg [128, 3, 1024] bf16: rows 0:96 = c, cols (d,o) d-major.
    WPg = const.tile([128, K, EMB * 64], BF16)
    nc.vector.memset(WPg[:], 0.0)
    WPu = const.tile([128, K, EMB * 32], BF16)
    nc.vector.memset(WPu[:], 0.0)
    WWg = const.tile([128, EMB * 64], BF16)
    nc.vector.memset(WWg[:], 0.0)
    WWu = const.tile([128, EMB * 32], BF16)
    nc.vector.memset(WWu[:], 0.0)

    Tb = const.tile([128, 2, WLEN], F32)
    for w, name in ((0, "gT"), (1, "uT")):
        src = io[name][:]
        nc.sync.dma_start(
            Tb[:, w, :],
            bass.AP(tensor=src.tensor, offset=src.offset, ap=[[0, 128]] + list(src.ap)),
        )

    X1 = big.tile([128, NCHUNK, B_LOC, 128], BF16, tag="slot1")
    X2 = big.tile([128, NCHUNK, B_LOC, 128], BF16, tag="slot2")
    X3 = big.tile([128, NCHUNK, B_LOC, 128], BF16, tag="slot3")
    nc.vector.memset(X1[:], 0.0)
    nc.gpsimd.memset(X2[:], 0.0)
    nc.gpsimd.memset(X3[:], 0.0)
    XtT = big.tile([128, B_LOC * NPAD], BF16, tag="XtT")
    r_gate = big.tile([128, NCHUNK, B_LOC, DOUT], BF16, tag="rgate")
    A = big.tile([128, NCHUNK, N], BF16, tag="A")

    FLAT = N * CW // 128  # 750

    with tc.tile_pool(name="stage", bufs=3) as stage, \
         tc.tile_pool(name="stage3", bufs=2) as stage3, \
         tc.tile_pool(name="xtacc", bufs=1) as xtacc:
        # ---- weight pools load/pack
        for k in range(K):
            wk = stage.tile([128, EMB, 64], F32, tag="stg")
            nc.sync.dma_start(wk[0:CIN], io["gw_pool"][:, k].rearrange("d c o -> c d o"))
            nc.vector.tensor_copy(
                WPg[0:CIN, k].rearrange("p (d o) -> p d o", d=EMB), wk[0:CIN]
            )
            wku = stage.tile([128, EMB, 32], F32, tag="stg")
            nc.sync.dma_start(wku[0:32], io["uw_pool"][:, k, 0:32, :].rearrange("d c o -> c d o"))
            nc.sync.dma_start(wku[64:128], io["uw_pool"][:, k, 32:96, :].rearrange("d c o -> c d o"))
            nc.vector.tensor_copy(
                WPu[0:32, k].rearrange("p (d o) -> p d o", d=EMB), wku[0:32]
            )
            nc.vector.tensor_copy(
                WPu[64:128, k].rearrange("p (d o) -> p d o", d=EMB), wku[64:128]
            )
        wg = stage.tile([128, EMB, 64], F32, tag="stg")
        nc.sync.dma_start(wg[0:CW], io["gw_win"].rearrange("d i o -> i d o"))
        nc.vector.tensor_copy(WWg[0:CW].rearrange("p (d o) -> p d o", d=EMB), wg[0:CW])
        wu = stage.tile([128, EMB, 32], F32, tag="stg")
        # rows 64:112 (matches packed XtT where xt_u.T sits at partitions 64:112)
        nc.sync.dma_start(wu[64 : 64 + CW], io["uw_win"].rearrange("d i o -> i d o"))
        nc.vector.tensor_copy(
            WWu[64 : 64 + CW].rearrange("p (d o) -> p d o", d=EMB), wu[64 : 64 + CW]
        )

        # ---- biases + A
        with tc.tile_pool(name="prep", bufs=1) as prep:
            embT_raw = prep.tile([EMB, N], F32)
            nc.sync.dma_start(embT_raw[:], emb.rearrange("n d -> d n"))
            embT = prep.tile([EMB, N], F32R)
            nc.vector.tensor_copy(embT[:], embT_raw[:])
            gbp_raw = prep.tile([EMB, 2 * DOUT], F32)
            nc.sync.dma_start(gbp_raw[:], io["gb_pool"][:])
            gbp_s = prep.tile([EMB, 2 * DOUT], F32R)
            nc.vector.tensor_copy(gbp_s[:], gbp_raw[:])
            ubp_raw = prep.tile([EMB, DOUT], F32)
            nc.sync.dma_start(ubp_raw[:], io["ub_pool"][:])
            ubp_s = prep.tile([EMB, DOUT], F32R)
            nc.vector.tensor_copy(ubp_s[:], ubp_raw[:])
            with tc.tile_pool(name="psum_pre", bufs=2, space="PSUM") as psum_pre:
                for nch in range(NCHUNK):
                    l = nlen(nch)
                    nsl = slice(nch * 128, nch * 128 + l)
                    pg = psum_pre.tile([128, N], F32, tag="pg")
                    for mj in range(4):
                        m0 = mj * 512
                        mw = min(512, N - m0)
                        nc.tensor.matmul(
                            pg[:l, m0 : m0 + mw], embT[:, nsl],
                            embT[:, m0 : m0 + mw], start=True, stop=True,
                        )
                    nc.scalar.activation(A[:l, nch, :], pg[:l, :], AF.Exp)
                    nc.vector.tensor_scalar(
                        out=A[:l, nch, :], in0=A[:l, nch, :],
                        scalar1=1.0, scalar2=0.0, op0=OP.max, op1=OP.add,
                        accum_out=dsum_all[:l, nch : nch + 1],
                    )
                    nc.vector.reciprocal(rinv[:l, nch : nch + 1], dsum_all[:l, nch : nch + 1])
            with tc.tile_pool(name="psum_b", bufs=2, space="PSUM") as psum_b:
                for nch in range(NCHUNK):
                    l = nlen(nch)
                    nsl = slice(nch * 128, nch * 128 + l)
                    pb = psum_b.tile([128, 3 * DOUT], F32, tag="pbias")
                    nc.tensor.matmul(
                        pb[:l, 0 : 2 * DOUT], embT[:, nsl],
                        gbp_s[:], start=True, stop=True,
                    )
                    nc.tensor.matmul(
                        pb[:l, 2 * DOUT :], embT[:, nsl],
                        ubp_s[:], start=True, stop=True,
                    )
                    nc.scalar.copy(biasg[:l, nch, :], pb[:l, 0 : 2 * DOUT])
                    nc.scalar.copy(biasu[:l, nch, :], pb[:l, 2 * DOUT :])

        # ---- x/state -> X1 [x | state] bf16
        for b in range(B_LOC):
            xs = stage.tile([128, NCHUNK, DIN], F32, tag="stg")
            nc.vector.memset(xs[64:, NCHUNK - 1], 0.0)
            chunked_load(nc, xs, x[b])
            nc.vector.tensor_copy(X1[:, :, b, 0:DIN], xs[:])
            ss = stage.tile([128, NCHUNK, DOUT], F32, tag="stg")
            nc.vector.memset(ss[64:, NCHUNK - 1], 0.0)
            chunked_load(nc, ss, state[b])
            nc.scalar.copy(X1[:, :, b, DIN:CIN], ss[:])

        # ---- window t-contraction (flat layout), then DRAM-bounce into XtT
        zeros128 = const.tile([128, 128], BF16)
        nc.vector.memset(zeros128[:], 0.0)
        dzero = dram.tile([NPAD, 128], BF16, tag="dzero")
        nc.sync.dma_start(
            dzero.rearrange("(c p) o -> p c o", p=128),
            bass.AP(tensor=zeros128.tensor, offset=zeros128.offset,
                    ap=[[1, 128], [0, NCHUNK], [1, 128]]),
        )
        xt_g = xtacc.tile([128, B_LOC, FLAT], F32)
        xt_u = xtacc.tile([128, B_LOC, FLAT], BF16)
        for t in range(WLEN):
            for b in range(B_LOC):
                st = stage3.tile([128, FLAT], F32, tag="xw")
                nc.sync.dma_start(
                    st[:],
                    x_full[b, t].rearrange("n i -> (n i)").rearrange("(p f) -> p f", p=128),
                )
                for w, acc in ((0, xt_g), (1, xt_u)):
                    if t == 0:
                        nc.vector.tensor_scalar(
                            out=acc[:, b, :], in0=st[:],
                            scalar1=Tb[:, w, 0:1], scalar2=None, op0=OP.mult,
                        )
                    else:
                        nc.vector.scalar_tensor_tensor(
                            out=acc[:, b, :], in0=st[:],
                            scalar=Tb[:, w, t : t + 1],
                            in1=acc[:, b, :], op0=OP.mult, op1=OP.add,
                        )
        # pack via DRAM: XtT partitions 0:48 = xt_g.T, 64:112 = xt_u.T
        for b in range(B_LOC):
            xgb16 = stage.tile([128, 2, FLAT], BF16, tag="stg")
            nc.gpsimd.tensor_copy(xgb16[:, 0, :], xt_g[:, b, :])
            nc.gpsimd.tensor_copy(xgb16[:, 1, :], xt_u[:, b, :])
            dflat = dram.tile([2, 128, FLAT], BF16, tag="dflat")
            nc.sync.dma_start(dflat.rearrange("w p f -> p w f"), xgb16[:])
            dpan = dram.tile([NPAD, 128], BF16, tag="pan")
            dfv = dflat.rearrange("w p f -> w (p f)").rearrange("w (n i) -> w n i", n=N)
            nc.sync.dma_start(dpan[0:N, 0:CW], dfv[0])
            nc.sync.dma_start(dpan[0:N, 64 : 64 + CW], dfv[1])
            nc.sync.dma_start(dpan[0:N, CW:64], dzero[0:N, 0:16])
            nc.sync.dma_start(dpan[0:N, 112:128], dzero[0:N, 0:16])
            nc.sync.dma_start(dpan[N:NPAD, :], dzero[N:NPAD, :])
            nc.sync.dma_start(XtT[:, b * NPAD : (b + 1) * NPAD], dpan[:], transpose=True)

        # ---- gate diffusion (A resident): nch-outer, psum accumulate over mi
        with tc.tile_pool(name="psum_d1", bufs=3, space="PSUM") as psum_d1:
            for SRC, DST in ((X1, X2), (X2, X3)):
                for nch in range(NCHUNK):
                    l = nlen(nch)
                    ph = psum_d1.tile([128, B_LOC, CIN], F32, tag="pdiff")
                    for mi in range(NCHUNK):
                        ml = nlen(mi)
                        nc.tensor.matmul(
                            ph[:l], A[:ml, mi, nch * 128 : nch * 128 + l],
                            SRC[:ml, mi, :, 0:CIN],
                            start=(mi == 0), stop=(mi == NCHUNK - 1),
                        )
                    nc.scalar.activation(
                        DST[:l, nch, :, 0:CIN], ph[:l],
                        AF.Copy, scale=rinv[:l, nch : nch + 1],
                    )
    # Apool/stage/xtacc closed: A + staging SBUF freed for the y phases.

    # ================= shared y-phase pools =================
    acc_pool = ctx.enter_context(tc.tile_pool(name="accp", bufs=2))
    tail_pool = ctx.enter_context(tc.tile_pool(name="tailp", bufs=2))
    ysu_pool = ctx.enter_context(tc.tile_pool(name="ysu", bufs=3))
    xtb_pool = ctx.enter_context(tc.tile_pool(name="xtb", bufs=2))

    def dred4(ysh, nblk, owid, nch, bias):
        """Batched d-reduction over all 4 b: returns acc [128, B_LOC, nblk*owid].

        ysh: two half-tiles [128, B_LOC, nblk*owid*8] bf16, ysh[h] holding
        y d-slices 8h..8h+7, per-b cols [blk0 8d x owid | blk1 8d x owid].
        bias: [128, nblk*owid] (seeded per-b on the DVE chain's d=0).
        """
        W = nblk * owid
        accA = acc_pool.tile([128, B_LOC, W], BF16, tag=f"accA{W}")
        ys = [t.rearrange("p b (blk d o) -> p b blk d o", blk=nblk, d=8)
              for t in ysh]
        bias3 = bias.rearrange("p (blk o) -> p blk o", blk=nblk)
        accA4 = accA.rearrange("p b (blk o) -> p b blk o", blk=nblk)
        for d in range(EMB):
            src = ys[d // 8][:, :, :, d % 8, :]
            if d == 0:
                for b in range(B_LOC):
                    nc.vector.scalar_tensor_tensor(
                        out=accA4[:, b], in0=src[:, b],
                        scalar=eexp[:, nch, 0:1],
                        in1=bias3[:], op0=OP.mult, op1=OP.add,
                    )
            else:
                nc.vector.scalar_tensor_tensor(
                    out=accA4[:], in0=src,
                    scalar=eexp[:, nch, d : d + 1],
                    in1=accA4[:], op0=OP.mult, op1=OP.add,
                )
        return accA

    # ================= gate y-GEMM (nch-outer, b-batched tail) =================
    with tc.tile_pool(name="ysg", bufs=2) as ysg_pool, \
         tc.tile_pool(name="psum_yg", bufs=2, space="PSUM") as psum_yg, \
         tc.tile_pool(name="psum_tg", bufs=2, space="PSUM") as psum_tg:
        for nch in range(NCHUNK):
            l = nlen(nch)
            # --- PE transposes: X1T/X2T/X3T for all 4 b into one psum batch
            pt = psum_tg.tile([128, 3 * B_LOC * 128], BF16, tag="pt")
            for ki, S in enumerate((X1, X2, X3)):
                for b in range(B_LOC):
                    j = ki * B_LOC + b
                    nc.tensor.transpose(
                        pt[:, j * 128 : (j + 1) * 128], S[:, nch, b, :], ident[:]
                    )
            xtb = xtb_pool.tile([128, 3, B_LOC, 128], BF16, tag="xtb")
            nc.vector.tensor_copy(xtb[:], pt[:])
            # --- y matmuls: per (b, half) psum [128, 1024]
            ysh0 = ysg_pool.tile([128, B_LOC, 1024], BF16, tag="ysg")
            ysh1 = ysg_pool.tile([128, B_LOC, 1024], BF16, tag="ysg")
            ysh = (ysh0, ysh1)
            for half in range(2):
                for b in range(B_LOC):
                    py = psum_yg.tile([128, 1024], F32, tag="pyg")
                    hs = slice(half * 512, half * 512 + 512)
                    for k in range(K):
                        nc.tensor.matmul(
                            py[:, 0:512], xtb[0:CIN, k, b, :],
                            WPg[0:CIN, k, hs],
                            start=(k == 0), stop=(k == K - 1),
                        )
                    nc.tensor.matmul(
                        py[:, 512:1024],
                        XtT[:, b * NPAD + nch * 128 : b * NPAD + nch * 128 + 128],
                        WWg[:, hs], start=True, stop=True,
                    )
                    # evict halves on two engines (f32 -> bf16)
                    nc.scalar.copy(ysh[half][:, b, 0:512], py[:, 0:512])
                    nc.vector.tensor_copy(ysh[half][:, b, 512:1024], py[:, 512:1024])
            # --- batched d-reduction + gating tail
            acc = dred4(ysh, 2, DOUT, nch, biasg[:, nch])
            acc4 = acc.rearrange("p b (blk o) -> p b blk o", blk=2)
            ztile = acc_pool.tile([128, B_LOC, DOUT], BF16, tag="ztile")
            nc.scalar.activation(ztile[:], acc4[:, :, 0, :], AF.Sigmoid)
            nc.scalar.activation(r_gate[:, nch], acc4[:, :, 1, :], AF.Sigmoid)
            # zs = z*state in two halves (state read in place from X1 cols
            # 32:96; each op's read range is disjoint from its write range,
            # op ordering handles the overlap across the two).
            nc.gpsimd.tensor_mul(
                X1[:, nch, :, 96:128], ztile[:, :, 32:64], X1[:, nch, :, 64:96]
            )
            nc.gpsimd.tensor_mul(
                X1[:, nch, :, 64:96], ztile[:, :, 0:32], X1[:, nch, :, 32:64]
            )

    CAND = X1  # panels now hold [x | state(stale) | z*state]

    # ================= update diffusion =================
    C2, C3 = X2, X3
    with tc.tile_pool(name="psum_d2", bufs=3, space="PSUM") as psum_d2:
        for SRC, DST in ((CAND, C2), (C2, C3)):
            for nch in range(NCHUNK):
                l = nlen(nch)
                ph = psum_d2.tile([128, B_LOC, DOUT], F32, tag="pdiff2")
                for mi in range(NCHUNK):
                    ml = nlen(mi)
                    nc.tensor.matmul(
                        ph[:l], A[:ml, mi, nch * 128 : nch * 128 + l],
                        SRC[:ml, mi, :, 64:128],
                        start=(mi == 0), stop=(mi == NCHUNK - 1),
                    )
                nc.scalar.activation(
                    DST[:l, nch, :, 64:128], ph[:l],
                    AF.Copy, scale=rinv[:l, nch : nch + 1],
                )

    # ================= update y-GEMM + output =================
    with tc.tile_pool(name="psum_yu", bufs=2, space="PSUM") as psum_yu, \
         tc.tile_pool(name="psum_tu", bufs=2, space="PSUM") as psum_tu:
        for nch in range(NCHUNK):
            l = nlen(nch)
            pt = psum_tu.tile([128, 3 * B_LOC * 128], BF16, tag="ptu")
            for ki, S in enumerate((CAND, C2, C3)):
                for b in range(B_LOC):
                    j = ki * B_LOC + b
                    nc.tensor.transpose(
                        pt[:, j * 128 : (j + 1) * 128], S[:, nch, b, :], ident[:]
                    )
            xtb = xtb_pool.tile([128, 3, B_LOC, 128], BF16, tag="xtb")
            nc.vector.tensor_copy(xtb[:], pt[:])
            ysh0 = ysu_pool.tile([128, B_LOC, 512], BF16, tag="ysu")
            ysh1 = ysu_pool.tile([128, B_LOC, 512], BF16, tag="ysu")
            ysh = (ysh0, ysh1)
            for b in range(B_LOC):
                pu = psum_yu.tile([128, 1024], F32, tag="pyu")
                for k in range(K):
                    nc.tensor.matmul(
                        pu[:, 0:512], xtb[:, k, b, :], WPu[:, k, :],
                        start=(k == 0), stop=(k == K - 1),
                    )
                nc.tensor.matmul(
                    pu[:, 512:1024],
                    XtT[:, b * NPAD + nch * 128 : b * NPAD + nch * 128 + 128],
                    WWu[:], start=True, stop=True,
                )
                # halves: graph d0:8 = pu[0:256], d8:16 = pu[256:512];
                # window d0:8 = pu[512:768], d8:16 = pu[768:1024]
                nc.scalar.copy(ysh0[:, b, 0:256], pu[:, 0:256])
                nc.vector.tensor_copy(ysh0[:, b, 256:512], pu[:, 512:768])
                nc.scalar.copy(ysh1[:, b, 0:256], pu[:, 256:512])
                nc.vector.tensor_copy(ysh1[:, b, 256:512], pu[:, 768:1024])
            accu = dred4(ysh, 2, 32, nch, biasu[:, nch])
            hc = tail_pool.tile([128, B_LOC, DOUT], F32, tag="hc")
            nc.scalar.activation(hc[:], accu[:], AF.Tanh)
            stf = tail_pool.tile([128, B_LOC, DOUT], F32, tag="stf")
            if l < 128:
                nc.vector.memset(stf[64:], 0.0)
            for b in range(B_LOC):
                nc.sync.dma_start(stf[:l, b], state[b, nch * 128 : nch * 128 + l, :])
            tmp = tail_pool.tile([128, B_LOC, DOUT], F32, tag="tmp")
            nc.vector.tensor_sub(tmp[:], stf[:], hc[:])
            nc.gpsimd.tensor_mul(tmp[:], tmp[:], r_gate[:, nch])
            nc.vector.tensor_add(tmp[:], tmp[:], hc[:])
            for b in range(B_LOC):
                nc.sync.dma_start(
                    out[b, nch * 128 : nch * 128 + l, :], tmp[:l, b, :]
                )


def make_io(nc):
    io = {}
    io["x"] = nc.dram_tensor("x", [B_LOC, N, DIN], F32, kind="ExternalInput")
    io["state"] = nc.dram_tensor("state", [B_LOC, N, DOUT], F32, kind="ExternalInput")
    io["x_full"] = nc.dram_tensor("x_full", [B_LOC, WLEN, N, CW], F32, kind="ExternalInput")
    io["node_embeddings"] = nc.dram_tensor("node_embeddings", [N, EMB], F32, kind="ExternalInput")
    io["gw_pool"] = nc.dram_tensor("gw_pool", [EMB, K, CIN, 64], F32, kind="ExternalInput")
    io["gw_win"] = nc.dram_tensor("gw_win", [EMB, CW, 64], F32, kind="ExternalInput")
    io["gb_pool"] = nc.dram_tensor("gb_pool", [EMB, 2 * DOUT], F32, kind="ExternalInput")
    io["gT"] = nc.dram_tensor("gT", [WLEN], F32, kind="ExternalInput")
    io["uw_pool"] = nc.dram_tensor("uw_pool", [EMB, K, CIN, 32], F32, kind="ExternalInput")
    io["uw_win"] = nc.dram_tensor("uw_win", [EMB, CW, 32], F32, kind="ExternalInput")
    io["ub_pool"] = nc.dram_tensor("ub_pool", [EMB, DOUT], F32, kind="ExternalInput")
    io["uT"] = nc.dram_tensor("uT", [WLEN], F32, kind="ExternalInput")
    io["out"] = nc.dram_tensor("out", [B_LOC, N, DOUT], F32, kind="ExternalOutput")
    return io


def build_module(debug=False):
    from concourse import bacc

    nc = bacc.Bacc("TRN2", target_bir_lowering=False, debug=debug)
    io = make_io(nc)
    with tile.TileContext(nc) as tc:
        build(tc, io)
    nc.finalize()
    return nc


# ======================= harness wrapper =======================
import numpy as _np

N_CORES = 8
_CACHE = {}


def _get_module():
    if "nc" not in _CACHE:
        _CACHE["nc"] = build_module()
    return _CACHE["nc"]


def make_in_maps(inputs):
    xb = _np.ascontiguousarray(inputs["x"], dtype=_np.float32)
    sb = _np.ascontiguousarray(inputs["state"], dtype=_np.float32)
    xf = _np.ascontiguousarray(inputs["x_full"], dtype=_np.float32)
    rep = {
        k: _np.ascontiguousarray(inputs[k], dtype=_np.float32)
        for k in ("node_embeddings", "gw_pool", "gw_win", "gb_pool", "gT",
                  "uw_pool", "uw_win", "ub_pool", "uT")
    }
    in_maps = []
    for i in range(N_CORES):
        m = dict(rep)
        m["x"] = xb[i * B_LOC : (i + 1) * B_LOC]
        m["state"] = sb[i * B_LOC : (i + 1) * B_LOC]
        m["x_full"] = xf[i * B_LOC : (i + 1) * B_LOC]
        in_maps.append(m)
    return in_maps


def kernel(**inputs):
    """Full-input entry point: shards over batch across 8 NeuronCores."""
    nc = _get_module()
    from concourse.bass_utils import run_bass_kernel_spmd

    in_maps = make_in_maps(inputs)
    res = run_bass_kernel_spmd(nc, in_maps, core_ids=list(range(N_CORES)))
    return _np.concatenate([res.results[i]["out"] for i in range(N_CORES)], axis=0)


# revision 24
# speedup vs baseline: 1.0502x; 1.0502x over previous
import sys
for _p in ('/opt/trn_rl_repo',):
    if _p not in sys.path:
        sys.path.insert(0, _p)

"""NLSGCRN cell Bass/Tile kernel for TRN2, batch-sharded SPMD over 8 cores.

Per-core shapes (b_loc = 4 batches):
  x [4,2000,32], state [4,2000,64], x_full [4,12,2000,48], emb [2000,16],
  pools gw/uw/gb/ub/gT/uT, out h [4,2000,64].

v2 structure (vs v1 baseline):
- y-phases loop nch-outer / b-inner: the per-node d-reduction batches all
  4 batches into one op (the e[n,d] scalar is per-partition and shared
  across b), split DVE (d < D_DVE, seeded with bias) / GPSIMD (rest).
- XkT stationaries via PE transposes (identity matmul) into bf16 PSUM,
  evicted by DVE, replacing v1's DRAM-bounce DMA transposes.
- A is spilled to DRAM at generation and its SBUF tile is freed after the
  gate hops; update diffusion streams A chunks back mi-outer with all 16
  output-chunk accumulators resident in PSUM (8 banks exactly).
- PSUM bank budget in y phases: y tiles [128,1024] f32 (2 banks) x2 bufs
  + transpose batch [128,1536] bf16 (2 banks) x2 bufs.
"""

from contextlib import ExitStack

import concourse.bass as bass
import concourse.tile as tile
from concourse import mybir
from concourse._compat import with_exitstack
from concourse.masks import make_identity

F32 = mybir.dt.float32
F32R = mybir.dt.float32r
BF16 = mybir.dt.bfloat16
AF = mybir.ActivationFunctionType
OP = mybir.AluOpType

B_LOC = 4
N = 2000
NCHUNK = 16           # ceil(2000/128)
NFULL = (NCHUNK - 1) * 128   # 1920
NPAD = NCHUNK * 128   # 2048
DIN, DOUT = 32, 64
CIN = 96
CW = 48
WLEN = 12
EMB = 16
K = 3

D_DVE = 8   # d-reduction: first D_DVE iterations on DVE, rest on GPSIMD


def nlen(nch):
    return 128 if nch < NCHUNK - 1 else N - NFULL  # last = 80


def chunked_load(nc, dst, src, eng=None):
    """dst [128, NCHUNK, ...inner] <- src [2000, ...inner] splitting rows."""
    eng = eng or nc.sync
    inner = src.shape[1:]
    eng.dma_start(
        dst[:, 0 : NCHUNK - 1],
        src[0:NFULL].rearrange(
            "(c p) " + " ".join(f"i{j}" for j in range(len(inner)))
            + " -> p c " + " ".join(f"i{j}" for j in range(len(inner))),
            p=128,
        ),
    )
    eng.dma_start(dst[0 : N - NFULL, NCHUNK - 1], src[NFULL:N])


@with_exitstack
def build(ctx: ExitStack, tc: tile.TileContext, io: dict):
    nc = tc.nc

    io = {k: (v[:] if not isinstance(v, bass.AP) else v) for k, v in io.items()}
    x, state, x_full = io["x"], io["state"], io["x_full"]
    emb = io["node_embeddings"]
    out = io["out"]

    const = ctx.enter_context(tc.tile_pool(name="const", bufs=1))
    big = ctx.enter_context(tc.tile_pool(name="big", bufs=1))
    dram = ctx.enter_context(tc.tile_pool(name="dram", bufs=6, space="DRAM"))

    # ================= constants / weights =================
    eexp = const.tile([128, NCHUNK, EMB], BF16)
    nc.vector.memset(eexp[:], 0.0)
    chunked_load(nc, eexp, emb, eng=nc.gpsimd)
    eexpf = const.tile([128, NCHUNK, EMB], F32)
    nc.vector.memset(eexpf[:], 0.0)
    chunked_load(nc, eexpf, emb, eng=nc.gpsimd)

    ident = const.tile([128, 128], BF16)
    make_identity(nc, ident)

    biasg = const.tile([128, NCHUNK, 2 * DOUT], BF16)
    biasu = const.tile([128, NCHUNK, DOUT], BF16)
    nc.vector.memset(biasg[64:, NCHUNK - 1], 0.0)
    nc.vector.memset(biasu[64:, NCHUNK - 1], 0.0)
    rinv = const.tile([128, NCHUNK], F32)
    dsum_all = const.tile([128, NCHUNK], F32)

    # WPg [128, 3, 1024] bf16: rows 0:96 = c, cols (d,o) d-major.
    WPg = const.tile([128, K, EMB * 64], BF16)
    nc.vector.memset(WPg[:], 0.0)
    WPu = const.tile([128, K, EMB * 32], BF16)
    nc.vector.memset(WPu[:], 0.0)
    WWg = const.tile([128, EMB * 64], BF16)
    nc.vector.memset(WWg[:], 0.0)
    WWu = const.tile([128, EMB * 32], BF16)
    nc.vector.memset(WWu[:], 0.0)

    Tb = const.tile([128, 2, WLEN], F32)
    for w, name in ((0, "gT"), (1, "uT")):
        src = io[name][:]
        nc.sync.dma_start(
            Tb[:, w, :],
            bass.AP(tensor=src.tensor, offset=src.offset, ap=[[0, 128]] + list(src.ap)),
        )

    X1 = big.tile([128, NCHUNK, B_LOC, 128], BF16, tag="slot1")
    X2 = big.tile([128, NCHUNK, B_LOC, 128], BF16, tag="slot2")
    X3 = big.tile([128, NCHUNK, B_LOC, 128], BF16, tag="slot3")
    nc.vector.memset(X1[:], 0.0)
    nc.gpsimd.memset(X2[:], 0.0)
    nc.gpsimd.memset(X3[:], 0.0)
    XtT = big.tile([128, B_LOC * NPAD], BF16, tag="XtT")
    r_gate = big.tile([128, NCHUNK, B_LOC, DOUT], BF16, tag="rgate")
    A = big.tile([128, NCHUNK, N], BF16, tag="A")

    FLAT = N * CW // 128  # 750

    with tc.tile_pool(name="stage", bufs=3) as stage, \
         tc.tile_pool(name="stage3", bufs=2) as stage3, \
         tc.tile_pool(name="xtacc", bufs=1) as xtacc:
        # ---- weight pools load/pack
        for k in range(K):
            wk = stage.tile([128, EMB, 64], F32, tag="stg")
            nc.sync.dma_start(wk[0:CIN], io["gw_pool"][:, k].rearrange("d c o -> c d o"))
            nc.vector.tensor_copy(
                WPg[0:CIN, k].rearrange("p (d o) -> p d o", d=EMB), wk[0:CIN]
            )
            wku = stage.tile([128, EMB, 32], F32, tag="stg")
            nc.sync.dma_start(wku[0:32], io["uw_pool"][:, k, 0:32, :].rearrange("d c o -> c d o"))
            nc.sync.dma_start(wku[64:128], io["uw_pool"][:, k, 32:96, :].rearrange("d c o -> c d o"))
            nc.vector.tensor_copy(
                WPu[0:32, k].rearrange("p (d o) -> p d o", d=EMB), wku[0:32]
            )
            nc.vector.tensor_copy(
                WPu[64:128, k].rearrange("p (d o) -> p d o", d=EMB), wku[64:128]
            )
        wg = stage.tile([128, EMB, 64], F32, tag="stg")
        nc.sync.dma_start(wg[0:CW], io["gw_win"].rearrange("d i o -> i d o"))
        nc.vector.tensor_copy(WWg[0:CW].rearrange("p (d o) -> p d o", d=EMB), wg[0:CW])
        wu = stage.tile([128, EMB, 32], F32, tag="stg")
        # rows 64:112 (matches packed XtT where xt_u.T sits at partitions 64:112)
        nc.sync.dma_start(wu[64 : 64 + CW], io["uw_win"].rearrange("d i o -> i d o"))
        nc.vector.tensor_copy(
            WWu[64 : 64 + CW].rearrange("p (d o) -> p d o", d=EMB), wu[64 : 64 + CW]
        )

        # ---- biases + A
        with tc.tile_pool(name="prep", bufs=1) as prep:
            embT_raw = prep.tile([EMB, N], F32)
            nc.sync.dma_start(embT_raw[:], emb.rearrange("n d -> d n"))
            embT = prep.tile([EMB, N], F32R)
            nc.vector.tensor_copy(embT[:], embT_raw[:])
            gbp_raw = prep.tile([EMB, 2 * DOUT], F32)
            nc.sync.dma_start(gbp_raw[:], io["gb_pool"][:])
            gbp_s = prep.tile([EMB, 2 * DOUT], F32R)
            nc.vector.tensor_copy(gbp_s[:], gbp_raw[:])
            ubp_raw = prep.tile([EMB, DOUT], F32)
            nc.sync.dma_start(ubp_raw[:], io["ub_pool"][:])
            ubp_s = prep.tile([EMB, DOUT], F32R)
            nc.vector.tensor_copy(ubp_s[:], ubp_raw[:])
            with tc.tile_pool(name="psum_pre", bufs=2, space="PSUM") as psum_pre:
                for nch in range(NCHUNK):
                    l = nlen(nch)
                    nsl = slice(nch * 128, nch * 128 + l)
                    pg = psum_pre.tile([128, N], F32, tag="pg")
                    for mj in range(4):
                        m0 = mj * 512
                        mw = min(512, N - m0)
                        nc.tensor.matmul(
                            pg[:l, m0 : m0 + mw], embT[:, nsl],
                            embT[:, m0 : m0 + mw], start=True, stop=True,
                        )
                    nc.scalar.activation(A[:l, nch, :], pg[:l, :], AF.Exp)
                    nc.vector.tensor_scalar(
                        out=A[:l, nch, :], in0=A[:l, nch, :],
                        scalar1=1.0, scalar2=0.0, op0=OP.max, op1=OP.add,
                        accum_out=dsum_all[:l, nch : nch + 1],
                    )
                    nc.vector.reciprocal(rinv[:l, nch : nch + 1], dsum_all[:l, nch : nch + 1])
            with tc.tile_pool(name="psum_b", bufs=2, space="PSUM") as psum_b:
                for nch in range(NCHUNK):
                    l = nlen(nch)
                    nsl = slice(nch * 128, nch * 128 + l)
                    pb = psum_b.tile([128, 3 * DOUT], F32, tag="pbias")
                    nc.tensor.matmul(
                        pb[:l, 0 : 2 * DOUT], embT[:, nsl],
                        gbp_s[:], start=True, stop=True,
                    )
                    nc.tensor.matmul(
                        pb[:l, 2 * DOUT :], embT[:, nsl],
                        ubp_s[:], start=True, stop=True,
                    )
                    nc.scalar.copy(biasg[:l, nch, :], pb[:l, 0 : 2 * DOUT])
                    nc.scalar.copy(biasu[:l, nch, :], pb[:l, 2 * DOUT :])

        # ---- x/state -> X1 [x | state] bf16
        for b in range(B_LOC):
            xs = stage.tile([128, NCHUNK, DIN], F32, tag="stg")
            nc.vector.memset(xs[64:, NCHUNK - 1], 0.0)
            chunked_load(nc, xs, x[b])
            nc.vector.tensor_copy(X1[:, :, b, 0:DIN], xs[:])
            ss = stage.tile([128, NCHUNK, DOUT], F32, tag="stg")
            nc.vector.memset(ss[64:, NCHUNK - 1], 0.0)
            chunked_load(nc, ss, state[b])
            nc.scalar.copy(X1[:, :, b, DIN:CIN], ss[:])

        # ---- window t-contraction (flat layout), then DRAM-bounce into XtT
        zeros128 = const.tile([128, 128], BF16)
        nc.vector.memset(zeros128[:], 0.0)
        dzero = dram.tile([NPAD, 128], BF16, tag="dzero")
        nc.sync.dma_start(
            dzero.rearrange("(c p) o -> p c o", p=128),
            bass.AP(tensor=zeros128.tensor, offset=zeros128.offset,
                    ap=[[1, 128], [0, NCHUNK], [1, 128]]),
        )
        xt_g = xtacc.tile([128, B_LOC, FLAT], F32)
        xt_u = xtacc.tile([128, B_LOC, FLAT], BF16)
        for t in range(WLEN):
            for b in range(B_LOC):
                st = stage3.tile([128, FLAT], F32, tag="xw")
                nc.sync.dma_start(
                    st[:],
                    x_full[b, t].rearrange("n i -> (n i)").rearrange("(p f) -> p f", p=128),
                )
                for w, acc in ((0, xt_g), (1, xt_u)):
                    if t == 0:
                        nc.vector.tensor_scalar(
                            out=acc[:, b, :], in0=st[:],
                            scalar1=Tb[:, w, 0:1], scalar2=None, op0=OP.mult,
                        )
                    else:
                        nc.vector.scalar_tensor_tensor(
                            out=acc[:, b, :], in0=st[:],
                            scalar=Tb[:, w, t : t + 1],
                            in1=acc[:, b, :], op0=OP.mult, op1=OP.add,
                        )
        # pack via DRAM: XtT partitions 0:48 = xt_g.T, 64:112 = xt_u.T
        for b in range(B_LOC):
            xgb16 = stage.tile([128, 2, FLAT], BF16, tag="stg")
            nc.gpsimd.tensor_copy(xgb16[:, 0, :], xt_g[:, b, :])
            nc.gpsimd.tensor_copy(xgb16[:, 1, :], xt_u[:, b, :])
            dflat = dram.tile([2, 128, FLAT], BF16, tag="dflat")
            nc.sync.dma_start(dflat.rearrange("w p f -> p w f"), xgb16[:])
            dpan = dram.tile([NPAD, 128], BF16, tag="pan")
            dfv = dflat.rearrange("w p f -> w (p f)").rearrange("w (n i) -> w n i", n=N)
            nc.sync.dma_start(dpan[0:N, 0:CW], dfv[0])
            nc.sync.dma_start(dpan[0:N, 64 : 64 + CW], dfv[1])
            nc.sync.dma_start(dpan[0:N, CW:64], dzero[0:N, 0:16])
            nc.sync.dma_start(dpan[0:N, 112:128], dzero[0:N, 0:16])
            nc.sync.dma_start(dpan[N:NPAD, :], dzero[N:NPAD, :])
            nc.sync.dma_start(XtT[:, b * NPAD : (b + 1) * NPAD], dpan[:], transpose=True)

        # ---- gate diffusion (A resident): nch-outer, psum accumulate over mi
        with tc.tile_pool(name="psum_d1", bufs=3, space="PSUM") as psum_d1:
            for SRC, DST in ((X1, X2), (X2, X3)):
                for nch in range(NCHUNK):
                    l = nlen(nch)
                    ph = psum_d1.tile([128, B_LOC, CIN], F32, tag="pdiff")
                    for mi in range(NCHUNK):
                        ml = nlen(mi)
                        nc.tensor.matmul(
                            ph[:l], A[:ml, mi, nch * 128 : nch * 128 + l],
                            SRC[:ml, mi, :, 0:CIN],
                            start=(mi == 0), stop=(mi == NCHUNK - 1),
                        )
                    nc.scalar.activation(
                        DST[:l, nch, :, 0:CIN], ph[:l],
                        AF.Copy, scale=rinv[:l, nch : nch + 1],
                    )
    # Apool/stage/xtacc closed: A + staging SBUF freed for the y phases.

    # ================= shared y-phase pools =================
    acc_pool = ctx.enter_context(tc.tile_pool(name="accp", bufs=2))
    tmp_pool = ctx.enter_context(tc.tile_pool(name="tmpp", bufs=1))
    tail_pool = ctx.enter_context(tc.tile_pool(name="tailp", bufs=2))
    ysu_pool = ctx.enter_context(tc.tile_pool(name="ysu", bufs=3))
    xtb_pool = ctx.enter_context(tc.tile_pool(name="xtb", bufs=2))

    def dred4(ysh, nblk, owid, nch, bias):
        """Batched d-reduction over all 4 b: returns acc [128, B_LOC, nblk*owid].

        ysh: two half-tiles [128, B_LOC, nblk*owid*8] bf16, ysh[h] holding
        y d-slices 8h..8h+7, per-b cols [blk0 8d x owid | blk1 8d x owid].
        bias: [128, nblk*owid] (seeded per-b on the DVE chain's d=0).
        """
        # Decomposed as 16 DVE tensor_scalar muls (4x-mode capable, unlike
        # scalar_tensor_tensor which has no DVE perf modes) + an add chain
        # split between DVE (accA: t0..t11 + per-b bias) and GPSIMD
        # (accB: t12..t15, then the accA+accB merge).
        W = nblk * owid
        N_POOL = 4  # trailing d-terms accumulated on GPSIMD
        accA = acc_pool.tile([128, B_LOC, W], BF16, tag=f"accA{W}")
        accB = acc_pool.tile([128, B_LOC, W], BF16, tag=f"accB{W}")
        ys = [t.rearrange("p b (blk d o) -> p b blk d o", blk=nblk, d=8)
              for t in ysh]
        bias3 = bias.rearrange("p (blk o) -> p blk o", blk=nblk)
        accA4 = accA.rearrange("p b (blk o) -> p b blk o", blk=nblk)
        t0 = tmp_pool.tile([128, B_LOC, W], BF16, tag=f"t0_{W}")
        t1 = tmp_pool.tile([128, B_LOC, W], BF16, tag=f"t1_{W}")
        t2 = tmp_pool.tile([128, B_LOC, W], BF16, tag=f"t2_{W}")
        tt = [t0, t1, t2]
        nd = EMB - N_POOL
        for d in range(EMB):
            src = ys[d // 8][:, :, :, d % 8, :]
            if d == 0:
                nc.vector.tensor_scalar(
                    out=accA[:], in0=src,
                    scalar1=eexpf[:, nch, 0:1], scalar2=None, op0=OP.mult,
                )
            elif d == nd:
                nc.vector.tensor_scalar(
                    out=accB[:], in0=src,
                    scalar1=eexpf[:, nch, d : d + 1], scalar2=None, op0=OP.mult,
                )
            else:
                t = tt[d % 3]
                nc.vector.tensor_scalar(
                    out=t[:], in0=src,
                    scalar1=eexpf[:, nch, d : d + 1], scalar2=None, op0=OP.mult,
                )
                eng = nc.vector if d < nd else nc.gpsimd
                eng.tensor_tensor(
                    out=accA[:] if d < nd else accB[:],
                    in0=accA[:] if d < nd else accB[:],
                    in1=t[:], op=OP.add,
                )
        for b in range(B_LOC):
            nc.vector.tensor_tensor(
                out=accA4[:, b], in0=accA4[:, b], in1=bias3[:], op=OP.add,
            )
        nc.gpsimd.tensor_tensor(out=accA[:], in0=accA[:], in1=accB[:], op=OP.add)
        return accA

    # ================= gate y-GEMM (nch-outer, b-batched tail) =================
    with tc.tile_pool(name="ysg", bufs=2) as ysg_pool, \
         tc.tile_pool(name="psum_yg", bufs=2, space="PSUM") as psum_yg, \
         tc.tile_pool(name="psum_tg", bufs=2, space="PSUM") as psum_tg:
        for nch in range(NCHUNK):
            l = nlen(nch)
            # --- PE transposes: X1T/X2T/X3T for all 4 b into one psum batch
            pt = psum_tg.tile([128, 3 * B_LOC * 128], BF16, tag="pt")
            for ki, S in enumerate((X1, X2, X3)):
                for b in range(B_LOC):
                    j = ki * B_LOC + b
                    nc.tensor.transpose(
                        pt[:, j * 128 : (j + 1) * 128], S[:, nch, b, :], ident[:]
                    )
            xtb = xtb_pool.tile([128, 3, B_LOC, 128], BF16, tag="xtb")
            nc.scalar.copy(xtb[:], pt[:])
            # --- y matmuls: per (b, half) psum [128, 1024]
            ysh0 = ysg_pool.tile([128, B_LOC, 1024], BF16, tag="ysg")
            ysh1 = ysg_pool.tile([128, B_LOC, 1024], BF16, tag="ysg")
            ysh = (ysh0, ysh1)
            for half in range(2):
                for b in range(B_LOC):
                    py = psum_yg.tile([128, 1024], F32, tag="pyg")
                    hs = slice(half * 512, half * 512 + 512)
                    for k in range(K):
                        nc.tensor.matmul(
                            py[:, 0:512], xtb[0:CIN, k, b, :],
                            WPg[0:CIN, k, hs],
                            start=(k == 0), stop=(k == K - 1),
                        )
                    nc.tensor.matmul(
                        py[:, 512:1024],
                        XtT[:, b * NPAD + nch * 128 : b * NPAD + nch * 128 + 128],
                        WWg[:, hs], start=True, stop=True,
                    )
                    nc.scalar.copy(ysh[half][:, b, 0:512], py[:, 0:512])
                    nc.scalar.copy(ysh[half][:, b, 512:1024], py[:, 512:1024])
            # --- batched d-reduction + gating tail
            acc = dred4(ysh, 2, DOUT, nch, biasg[:, nch])
            acc4 = acc.rearrange("p b (blk o) -> p b blk o", blk=2)
            ztile = acc_pool.tile([128, B_LOC, DOUT], BF16, tag="ztile")
            nc.scalar.activation(ztile[:], acc4[:, :, 0, :], AF.Sigmoid)
            nc.scalar.activation(r_gate[:, nch], acc4[:, :, 1, :], AF.Sigmoid)
            # zs = z*state in two halves (state read in place from X1 cols
            # 32:96; each op's read range is disjoint from its write range,
            # op ordering handles the overlap across the two).
            nc.gpsimd.tensor_mul(
                X1[:, nch, :, 96:128], ztile[:, :, 32:64], X1[:, nch, :, 64:96]
            )
            nc.gpsimd.tensor_mul(
                X1[:, nch, :, 64:96], ztile[:, :, 0:32], X1[:, nch, :, 32:64]
            )

    CAND = X1  # panels now hold [x | state(stale) | z*state]

    # ================= update diffusion =================
    C2, C3 = X2, X3
    with tc.tile_pool(name="psum_d2", bufs=3, space="PSUM") as psum_d2:
        for SRC, DST in ((CAND, C2), (C2, C3)):
            for nch in range(NCHUNK):
                l = nlen(nch)
                ph = psum_d2.tile([128, B_LOC, DOUT], F32, tag="pdiff2")
                for mi in range(NCHUNK):
                    ml = nlen(mi)
                    nc.tensor.matmul(
                        ph[:l], A[:ml, mi, nch * 128 : nch * 128 + l],
                        SRC[:ml, mi, :, 64:128],
                        start=(mi == 0), stop=(mi == NCHUNK - 1),
                    )
                nc.scalar.activation(
                    DST[:l, nch, :, 64:128], ph[:l],
                    AF.Copy, scale=rinv[:l, nch : nch + 1],
                )

    # ================= update y-GEMM + output =================
    with tc.tile_pool(name="psum_yu", bufs=2, space="PSUM") as psum_yu, \
         tc.tile_pool(name="psum_tu", bufs=2, space="PSUM") as psum_tu:
        for nch in range(NCHUNK):
            l = nlen(nch)
            pt = psum_tu.tile([128, 3 * B_LOC * 128], BF16, tag="ptu")
            for ki, S in enumerate((CAND, C2, C3)):
                for b in range(B_LOC):
                    j = ki * B_LOC + b
                    nc.tensor.transpose(
                        pt[:, j * 128 : (j + 1) * 128], S[:, nch, b, :], ident[:]
                    )
            xtb = xtb_pool.tile([128, 3, B_LOC, 128], BF16, tag="xtb")
            nc.scalar.copy(xtb[:], pt[:])
            ysh0 = ysu_pool.tile([128, B_LOC, 512], BF16, tag="ysu")
            ysh1 = ysu_pool.tile([128, B_LOC, 512], BF16, tag="ysu")
            ysh = (ysh0, ysh1)
            for b in range(B_LOC):
                pu = psum_yu.tile([128, 1024], F32, tag="pyu")
                for k in range(K):
                    nc.tensor.matmul(
                        pu[:, 0:512], xtb[:, k, b, :], WPu[:, k, :],
                        start=(k == 0), stop=(k == K - 1),
                    )
                nc.tensor.matmul(
                    pu[:, 512:1024],
                    XtT[:, b * NPAD + nch * 128 : b * NPAD + nch * 128 + 128],
                    WWu[:], start=True, stop=True,
                )
                # halves: graph d0:8 = pu[0:256], d8:16 = pu[256:512];
                # window d0:8 = pu[512:768], d8:16 = pu[768:1024]
                nc.scalar.copy(ysh0[:, b, 0:256], pu[:, 0:256])
                nc.scalar.copy(ysh0[:, b, 256:512], pu[:, 512:768])
                nc.scalar.copy(ysh1[:, b, 0:256], pu[:, 256:512])
                nc.scalar.copy(ysh1[:, b, 256:512], pu[:, 768:1024])
            accu = dred4(ysh, 2, 32, nch, biasu[:, nch])
            hc = tail_pool.tile([128, B_LOC, DOUT], F32, tag="hc")
            nc.scalar.activation(hc[:], accu[:], AF.Tanh)
            stf = tail_pool.tile([128, B_LOC, DOUT], F32, tag="stf")
            if l < 128:
                nc.vector.memset(stf[64:], 0.0)
            for b in range(B_LOC):
                nc.sync.dma_start(stf[:l, b], state[b, nch * 128 : nch * 128 + l, :])
            tmp = tail_pool.tile([128, B_LOC, DOUT], F32, tag="tmp")
            nc.vector.tensor_sub(tmp[:], stf[:], hc[:])
            nc.gpsimd.tensor_mul(tmp[:], tmp[:], r_gate[:, nch])
            nc.vector.tensor_add(tmp[:], tmp[:], hc[:])
            for b in range(B_LOC):
                nc.sync.dma_start(
                    out[b, nch * 128 : nch * 128 + l, :], tmp[:l, b, :]
                )


def make_io(nc):
    io = {}
    io["x"] = nc.dram_tensor("x", [B_LOC, N, DIN], F32, kind="ExternalInput")
    io["state"] = nc.dram_tensor("state", [B_LOC, N, DOUT], F32, kind="ExternalInput")
    io["x_full"] = nc.dram_tensor("x_full", [B_LOC, WLEN, N, CW], F32, kind="ExternalInput")
    io["node_embeddings"] = nc.dram_tensor("node_embeddings", [N, EMB], F32, kind="ExternalInput")
    io["gw_pool"] = nc.dram_tensor("gw_pool", [EMB, K, CIN, 64], F32, kind="ExternalInput")
    io["gw_win"] = nc.dram_tensor("gw_win", [EMB, CW, 64], F32, kind="ExternalInput")
    io["gb_pool"] = nc.dram_tensor("gb_pool", [EMB, 2 * DOUT], F32, kind="ExternalInput")
    io["gT"] = nc.dram_tensor("gT", [WLEN], F32, kind="ExternalInput")
    io["uw_pool"] = nc.dram_tensor("uw_pool", [EMB, K, CIN, 32], F32, kind="ExternalInput")
    io["uw_win"] = nc.dram_tensor("uw_win", [EMB, CW, 32], F32, kind="ExternalInput")
    io["ub_pool"] = nc.dram_tensor("ub_pool", [EMB, DOUT], F32, kind="ExternalInput")
    io["uT"] = nc.dram_tensor("uT", [WLEN], F32, kind="ExternalInput")
    io["out"] = nc.dram_tensor("out", [B_LOC, N, DOUT], F32, kind="ExternalOutput")
    return io


def build_module(debug=False):
    from concourse import bacc

    nc = bacc.Bacc("TRN2", target_bir_lowering=False, debug=debug)
    io = make_io(nc)
    with tile.TileContext(nc) as tc:
        build(tc, io)
    nc.finalize()
    return nc


# ======================= harness wrapper =======================
import numpy as _np

N_CORES = 8
_CACHE = {}


def _get_module():
    if "nc" not in _CACHE:
        _CACHE["nc"] = build_module()
    return _CACHE["nc"]


def make_in_maps(inputs):
    xb = _np.ascontiguousarray(inputs["x"], dtype=_np.float32)
    sb = _np.ascontiguousarray(inputs["state"], dtype=_np.float32)
    xf = _np.ascontiguousarray(inputs["x_full"], dtype=_np.float32)
    rep = {
        k: _np.ascontiguousarray(inputs[k], dtype=_np.float32)
        for k in ("node_embeddings", "gw_pool", "gw_win", "gb_pool", "gT",
                  "uw_pool", "uw_win", "ub_pool", "uT")
    }
    in_maps = []
    for i in range(N_CORES):
        m = dict(rep)
        m["x"] = xb[i * B_LOC : (i + 1) * B_LOC]
        m["state"] = sb[i * B_LOC : (i + 1) * B_LOC]
        m["x_full"] = xf[i * B_LOC : (i + 1) * B_LOC]
        in_maps.append(m)
    return in_maps


def kernel(**inputs):
    """Full-input entry point: shards over batch across 8 NeuronCores."""
    nc = _get_module()
    from concourse.bass_utils import run_bass_kernel_spmd

    in_maps = make_in_maps(inputs)
    res = run_bass_kernel_spmd(nc, in_maps, core_ids=list(range(N_CORES)))
    return _np.concatenate([res.results[i]["out"] for i in range(N_CORES)], axis=0)
